# revision 12
# baseline (speedup 1.0000x reference)
"""Trainium2 Bass kernel for a single causal attention head (prefill).

Problem shapes (hardcoded): x [4, 4096, 2048], W_Q/W_K/W_V [2048, 128].
reference: Q = rope(x@W_Q), K = x@W_K, V = rope(x@W_V),
           out = softmax(causal(Q K^T / sqrt(128))) @ V.

Sharding: 8 cores = (batch b, stripe h in {0,1}).  Core (b,h) computes the
output rows of batch b belonging to the interleaved 128-row blocks
g = 2j + h (j = 0..15) — balancing the causal triangle between the two
cores of a batch.  Each core computes the full K/V of its batch locally
(no collectives; a pairwise AllGather measured ~62us latency here, far too
slow to be useful).

On-chip layout: everything transposed.  The host passes x^T per batch, so
projections contract over E with W e-tiles stationary and x^T moving,
producing Q^T/K^T/V^T [Dh=128 part, tok free] directly.  RoPE pairs are
made partition-contiguous by permuting W columns (even dims first) on the
host.  Scores are computed transposed (S^T[k, q]) in bf16; softmax skips
the max subtraction (scores are gaussian-bounded, exp stays in fp32
range); exp runs on ACT writing bf16; P^T @ V and the softmax denominator
(ones-matmul) accumulate in a single PSUM bank per 256-row q-pair; the
output is produced transposed in bf16 and rearranged on the host.

Emission interleaves attention pair w right behind projection window w so
every engine's in-order stream pipelines across the two phases.  Engine
budget per core (Tensor is the roofline at ~101us of warm matmul): ACT
does exp + the PSUM->SBUF bf16 staging copies (K^T, V^T, Q^T); GpSimd the
4 V-rope multiplies; DVE Q-rope, V-rope add/sub, the P^T transpose copy,
the 3-region causal-mask fixup of the diagonal k-tile and the output
divide.  All rope math is bf16 (2x DVE mode).

SPMD trick for the h stripe: both cores run one program that always
gathers q columns at window offsets {0, 256}.  For h=1 cores the host
rotates every 512-token window of x^T (and the rope tables) left by 128
columns, so those offsets select the h=1 blocks; K/V column order inside a
window changes with it, which only matters for the causal mask of the
diagonal k-tile.  In the rotated order the two partially-masked subtiles
share ONE triangle mask for both stripes, and subtile 3 is all-or-nothing
per stripe (a per-partition scalar flag).
"""

import os
import sys

for _p in (
    "/root/.axon_site",
    "/root/.axon_site/_ro/trn_rl_repo",
    "/root/.axon_site/_ro/pypackages",
    "/opt/trn_rl_repo",
):
    if os.path.isdir(_p) and _p not in sys.path:
        sys.path.append(_p)

import numpy as np
import ml_dtypes

import concourse.mybir as mybir
import concourse.tile as tile
from concourse import bacc
from concourse.bass_utils import run_bass_kernel_spmd
from concourse.masks import make_identity

B, S, E, DH = 4, 4096, 2048, 128
NE = E // 128            # 16 e-tiles
NW = 8                   # windows of 512 tokens
WTOK = S // NW           # 512
NPAIR = 8                # q pairs per core (each 256 q rows)
NKSUB = S // 128         # 32 global k sub-blocks of 128
SCALE = 1.0 / float(np.sqrt(np.float64(DH)))
F32, BF16 = mybir.dt.float32, mybir.dt.bfloat16
ACT_EXP = mybir.ActivationFunctionType.Exp
ACT_COPY = mybir.ActivationFunctionType.Copy

_CACHE = {}


def _build():
    nc = bacc.Bacc()

    xt = nc.dram_tensor("xt", [NW, 128, NE, WTOK], BF16, kind="ExternalInput")
    wq = nc.dram_tensor("wq", [128, NE, DH], BF16, kind="ExternalInput")
    wk = nc.dram_tensor("wk", [128, NE, DH], BF16, kind="ExternalInput")
    wv = nc.dram_tensor("wv", [128, NE, DH], BF16, kind="ExternalInput")
    cosv = nc.dram_tensor("cosv", [128, S], BF16, kind="ExternalInput")
    sinv = nc.dram_tensor("sinv", [128, S], BF16, kind="ExternalInput")
    # [:,0,:] lower-triangle (r<=i) shared by both stripes; [:,1,:] stripe
    # flag broadcast (0.0 for h=0, 1.0 for h=1)
    maskt = nc.dram_tensor("maskt", [128, 2, 128], BF16, kind="ExternalInput")
    hflag = nc.dram_tensor("hflag", [128, 1], F32, kind="ExternalInput")
    outt = nc.dram_tensor("outt", [128, 16, 128], BF16, kind="ExternalOutput")

    with tile.TileContext(nc) as tc:
        with (
            tc.tile_pool(name="consts", bufs=1) as consts,
            tc.tile_pool(name="big", bufs=1) as big,
            tc.tile_pool(name="xs", bufs=5) as xs,
            tc.tile_pool(name="stage", bufs=2) as stage,
            tc.tile_pool(name="rope", bufs=2) as rope,
            tc.tile_pool(name="pt", bufs=6) as ptp,
            tc.tile_pool(name="fin", bufs=4) as fin,
            tc.tile_pool(name="ppsum", bufs=2, space="PSUM") as ppsum,
            tc.tile_pool(name="spsum", bufs=2, space="PSUM") as spsum,
            tc.tile_pool(name="apsum", bufs=1, space="PSUM") as apsum,
        ):
            # -- startup: first matmul needs wk e0/e1 + x w0 e0/e1 only --
            w_sb = {}
            for name, dram in (("wk", wk), ("wv", wv), ("wq", wq)):
                w_sb[name] = consts.tile(
                    [128, NE, DH], BF16, tag=name, name=name
                )
            nc.scalar.dma_start(out=w_sb["wk"][:, 0:2, :], in_=wk[:, 0:2, :])

            xc0 = xs.tile([128, NE, WTOK], BF16, tag="xc", name="xc_0")
            for q8 in range(8):
                nc.sync.dma_start(
                    out=xc0[:, 2 * q8 : 2 * q8 + 2, :],
                    in_=xt[0, :, 2 * q8 : 2 * q8 + 2, :],
                )

            nc.scalar.dma_start(out=w_sb["wk"][:, 2:NE, :], in_=wk[:, 2:NE, :])
            nc.scalar.dma_start(out=w_sb["wv"], in_=wv[:, :, :])
            nc.scalar.dma_start(out=w_sb["wq"], in_=wq[:, :, :])
            mask_sb = consts.tile([128, 2, 128], BF16, tag="maskt")
            nc.scalar.dma_start(out=mask_sb, in_=maskt[:, :, :])
            hflag_sb = consts.tile([128, 1], F32, tag="hflag")
            nc.scalar.dma_start(out=hflag_sb, in_=hflag[:, :])

            ident = consts.tile([128, 128], BF16, tag="ident")
            make_identity(nc, ident)

            kt_sb = big.tile([128, S], BF16, tag="kt")
            qt_sb = big.tile([128, 2048], BF16, tag="qt")
            vn_sb = big.tile([128, NKSUB, 132], BF16, tag="vn")
            nc.vector.memset(vn_sb[:, :, 128:129], 1.0)
            out_sb = big.tile([128, 16, 128], BF16, tag="outt")

            def project_window(w, xc):
                csw = stage.tile([128, WTOK], BF16, tag="csw", name=f"csw_{w}")
                snw = stage.tile([128, WTOK], BF16, tag="snw", name=f"snw_{w}")
                nc.sync.dma_start(out=csw, in_=cosv[:, w * WTOK : (w + 1) * WTOK])
                nc.sync.dma_start(out=snw, in_=sinv[:, w * WTOK : (w + 1) * WTOK])

                # K projection: K^T [Dh, 512] for this window
                kp = ppsum.tile([128, WTOK], F32, tag="proj", name=f"kp_{w}")
                for e in range(NE):
                    nc.tensor.matmul(
                        kp, w_sb["wk"][:, e, :], xc[:, e, :],
                        start=(e == 0), stop=(e == NE - 1),
                    )
                nc.scalar.activation(
                    kt_sb[:, w * WTOK : (w + 1) * WTOK], kp, ACT_COPY,
                    bias=0.0, scale=1.0,
                )

                # V projection: V^T [Dh, 512], stage to bf16, rope,
                # transpose to natural layout
                vp = ppsum.tile([128, WTOK], F32, tag="proj", name=f"vp_{w}")
                for e in range(NE):
                    nc.tensor.matmul(
                        vp, w_sb["wv"][:, e, :], xc[:, e, :],
                        start=(e == 0), stop=(e == NE - 1),
                    )
                vb = stage.tile([128, WTOK], BF16, tag="vb", name=f"vb_{w}")
                nc.scalar.activation(vb, vp, ACT_COPY, bias=0.0, scale=1.0)
                vt = stage.tile([128, WTOK], BF16, tag="vt", name=f"vt_{w}")
                t1 = rope.tile([64, WTOK], BF16, tag="t1")
                t2 = rope.tile([64, WTOK], BF16, tag="t2")
                t3 = rope.tile([64, WTOK], BF16, tag="t3")
                t4 = rope.tile([64, WTOK], BF16, tag="t4")
                nc.vector.tensor_mul(t1, vb[0:64, :], csw[0:64, :])
                nc.vector.tensor_mul(t2, vb[64:128, :], snw[64:128, :])
                nc.vector.tensor_sub(vt[0:64, :], t1, t2)
                nc.vector.tensor_mul(t3, vb[0:64, :], snw[0:64, :])
                nc.vector.tensor_mul(t4, vb[64:128, :], csw[64:128, :])
                nc.vector.tensor_add(vt[64:128, :], t3, t4)

                # Q projection for pair w: q columns at window offsets
                # {0, 256} (see module docstring for the h=1 rotation trick)
                qp = ppsum.tile([128, 256], F32, tag="proj", name=f"qp_{w}")
                for e in range(NE):
                    rhs = xc[:, e, :].rearrange(
                        "p (a t b) -> p a t b", a=2, t=2
                    )[:, :, 0:1, :]
                    nc.tensor.matmul(
                        qp, w_sb["wq"][:, e, :], rhs,
                        start=(e == 0), stop=(e == NE - 1),
                    )
                qcs = csw[0:64, :].rearrange(
                    "p (a t b) -> p a t b", a=2, t=2
                )[:, :, 0:1, :]
                qsn = snw[0:64, :].rearrange(
                    "p (a t b) -> p a t b", a=2, t=2
                )[:, :, 0:1, :]
                q1 = rope.tile([64, 256], BF16, tag="q1")
                q2 = rope.tile([64, 256], BF16, tag="q2")
                q3 = rope.tile([64, 256], BF16, tag="q3")
                q4 = rope.tile([64, 256], BF16, tag="q4")
                qdst = qt_sb[:, 256 * w : 256 * (w + 1)]
                nc.vector.tensor_mul(q1, qp[0:64, :], qcs)
                nc.vector.tensor_mul(q2, qp[64:128, :], qsn)
                nc.vector.tensor_sub(qdst[0:64, :], q1, q2)
                nc.vector.tensor_mul(q3, qp[0:64, :], qsn)
                nc.vector.tensor_mul(q4, qp[64:128, :], qcs)
                nc.vector.tensor_add(qdst[64:128, :], q3, q4)

                # V transpose to natural layout, emitted after Q so the
                # rope chain latency hides behind the Q matmuls
                vtr = ppsum.tile([128, 4, 128], BF16, tag="proj", name=f"vtr_{w}")
                for s in range(4):
                    nc.tensor.transpose(
                        vtr[:, s, :], vt[:, s * 128 : (s + 1) * 128], ident
                    )
                nc.vector.tensor_copy(vn_sb[:, 4 * w : 4 * w + 4, 0:128], vtr)

            # Diagonal-tile mask geometry in the rotated window order (same
            # for both stripes except subtile 3):
            #   subtile 0 vs q-block 0 : triangle (r <= i)
            #   subtiles 1,2 vs q-block 0 : all-zero  -> skipped in AV
            #   subtiles 0,1 vs q-block 1 : all-valid
            #   subtile 2 vs q-block 1 : triangle (r <= i)
            #   subtile 3 vs both      : all-zero (h=0) / all-valid (h=1)
            def attend_pair(t):
                acc0 = apsum.tile([128, 132], F32, tag="acc0", name=f"acc0_{t}")
                acc1 = apsum.tile([128, 132], F32, tag="acc1", name=f"acc1_{t}")
                accs = [acc0, acc1]
                n_mm = 4 * (t + 1)
                mi = 0
                for kt in range(t + 1):
                    st = spsum.tile(
                        [128, 4, 256], F32, tag="st", name=f"st_{t}_{kt}"
                    )
                    for i in range(4):
                        g = 4 * kt + i
                        nc.tensor.matmul(
                            st[:, i, :],
                            kt_sb[:, g * 128 : (g + 1) * 128],
                            qt_sb[:, 256 * t : 256 * (t + 1)],
                            start=True, stop=True,
                        )
                    praw = ptp.tile(
                        [128, 4, 256], BF16, tag="praw", name=f"pr_{t}_{kt}"
                    )
                    nc.scalar.activation(
                        praw, st, ACT_EXP, bias=0.0, scale=1.0,
                    )
                    if kt == t:
                        ptm = ptp.tile([128, 4, 256], BF16, tag="ptm")
                        nc.vector.tensor_mul(
                            ptm[:, 0, 0:128], praw[:, 0, 0:128], mask_sb[:, 0, :]
                        )
                        nc.vector.tensor_mul(
                            ptm[:, 2, 128:256], praw[:, 2, 128:256],
                            mask_sb[:, 0, :],
                        )
                        nc.vector.tensor_single_scalar(
                            ptm[:, 3, :], praw[:, 3, :], hflag_sb[:, 0:1],
                            mybir.AluOpType.mult,
                        )

                    for i in range(4):
                        g = 4 * kt + i
                        first = mi == 0
                        last = mi == n_mm - 1
                        for blk in range(2):
                            if kt == t and blk == 0 and i in (1, 2):
                                # fully causal-masked for BOTH stripes
                                continue
                            if kt == t and (
                                i == 3
                                or (blk == 0 and i == 0)
                                or (blk == 1 and i == 2)
                            ):
                                src_ap = ptm[:, i, blk * 128 : (blk + 1) * 128]
                            else:
                                src_ap = praw[:, i, blk * 128 : (blk + 1) * 128]
                            nc.tensor.matmul(
                                accs[blk][:, 0:129],
                                src_ap,
                                vn_sb[:, g, 0:129],
                                start=first, stop=last,
                            )
                        mi += 1

                for blk in range(2):
                    recip = fin.tile([128, 1], F32, tag="recip")
                    nc.vector.reciprocal(recip, accs[blk][:, 128:129])
                    nc.vector.tensor_scalar_mul(
                        out_sb[:, 2 * t + blk, :], accs[blk][:, 0:128], recip
                    )
                    nc.sync.dma_start(
                        out=outt[:, 2 * t + blk, :],
                        in_=out_sb[:, 2 * t + blk, :],
                    )

            # interleaved emission: attention pair w follows window w so the
            # in-order engine streams pipeline across phases
            for w in range(NW):
                if w == 0:
                    xc = xc0
                else:
                    xc = xs.tile([128, NE, WTOK], BF16, tag="xc", name=f"xc_{w}")
                    for c in range(4):
                        nc.sync.dma_start(
                            out=xc[:, 4 * c : 4 * c + 4, :],
                            in_=xt[w, :, 4 * c : 4 * c + 4, :],
                        )
                project_window(w, xc)
                attend_pair(w)

    nc.compile()
    return nc


def _rope_tables():
    p = np.arange(64, dtype=np.float64)
    inv = 10000.0 ** (-2.0 * p / DH)
    pos = np.arange(S, dtype=np.float64)
    theta = inv[:, None] * pos[None, :]
    return (np.cos(theta).astype(np.float32), np.sin(theta).astype(np.float32))


def _mask_compact(h):
    """[:,0,:] = lower triangle r<=i (shared by both stripes); [:,1,:] =
    stripe flag (0/1) for diagonal subtile 3."""
    m = np.zeros((128, 2, 128), dtype=np.float32)
    r = np.arange(128)[:, None]
    i = np.arange(128)[None, :]
    m[:, 0, :] = r <= i
    m[:, 1, :] = float(h)
    return m.astype(ml_dtypes.bfloat16)


def _bf(a):
    return np.ascontiguousarray(a.astype(ml_dtypes.bfloat16))


def kernel(x, W_Q, W_K, W_V):
    x = np.asarray(x, dtype=np.float32)
    W_Q = np.asarray(W_Q, dtype=np.float32)
    W_K = np.asarray(W_K, dtype=np.float32)
    W_V = np.asarray(W_V, dtype=np.float32)

    if "nc" not in _CACHE:
        _CACHE["nc"] = _build()
    nc = _CACHE["nc"]

    perm = np.concatenate([np.arange(0, DH, 2), np.arange(1, DH, 2)])
    cos_t, sin_t = _rope_tables()

    wq_h = _bf(
        (W_Q * SCALE)[:, perm].reshape(NE, 128, DH).transpose(1, 0, 2)
    )
    wk_h = _bf(W_K[:, perm].reshape(NE, 128, DH).transpose(1, 0, 2))
    wv_h = _bf(W_V[:, perm].reshape(NE, 128, DH).transpose(1, 0, 2))

    cos_rot = np.roll(cos_t.reshape(64, NW, WTOK), -128, axis=2).reshape(64, S)
    sin_rot = np.roll(sin_t.reshape(64, NW, WTOK), -128, axis=2).reshape(64, S)
    dup = lambda a: np.concatenate([a, a], axis=0)
    cos_b, sin_b = _bf(dup(cos_t)), _bf(dup(sin_t))
    cos_rot_b, sin_rot_b = _bf(dup(cos_rot)), _bf(dup(sin_rot))
    masks = [_mask_compact(0), _mask_compact(1)]

    in_maps = []
    metas = []
    for b in range(B):
        xt_b = x[b].T.reshape(NE, 128, NW, WTOK)
        for h in range(2):
            if h == 0:
                xt_c = _bf(xt_b.transpose(2, 1, 0, 3))
                cos_c, sin_c = cos_b, sin_b
            else:
                rot = np.roll(xt_b, -128, axis=3)
                xt_c = _bf(rot.transpose(2, 1, 0, 3))
                cos_c, sin_c = cos_rot_b, sin_rot_b
            in_maps.append(
                {
                    "xt": xt_c,
                    "wq": wq_h,
                    "wk": wk_h,
                    "wv": wv_h,
                    "cosv": cos_c,
                    "sinv": sin_c,
                    "maskt": masks[h],
                    "hflag": np.full((128, 1), float(h), dtype=np.float32),
                }
            )
            metas.append((b, h))

    global _LAST_IN_MAPS
    _LAST_IN_MAPS = in_maps

    try:
        res = run_bass_kernel_spmd(nc, in_maps, list(range(8)))
    except Exception:
        # transient NRT device errors have been observed; retry once
        import time as _time

        _time.sleep(2.0)
        res = run_bass_kernel_spmd(nc, in_maps, list(range(8)))

    out = np.empty((B, S, DH), dtype=np.float32)
    for c, (b, h) in enumerate(metas):
        ot = res.results[c]["outt"].astype(np.float32)
        for j in range(16):
            g = 2 * j + h
            out[b, g * 128 : (g + 1) * 128, :][:, perm] = ot[:, j, :]
    return out
